# revision 25
# baseline (speedup 1.0000x reference)
"""BitLinear FFN (BitNet b1.58) Trainium2 kernel, 8-core SPMD — v2.

Strategy: data-parallel over tokens (1024 tokens/core). Host passes
pre-transposed inputs (xT, wgT, wuT, wdT) so no PE transposes are needed.
Weight quantization is sharded 1/8 per core (two streamed passes: |w| sums,
then quantize->fp8 into AllGather staging). Three pipelined AllGathers
(wg -> wu -> wd) overlap the main loop: chunk-0 gate matmuls start once wg
lands; the up phase covers the wu AG; the down phase covers the wd AG.

Exactness: act quant produces ints in [-127,127] (exact bf16); weights are
ternary fp8e4. PE accumulates fp32 -> integer-exact matmuls. The silu'd
gate and the gate*up product are carried in fp16 (rel err ~2^-11, well under
the 2e-2 gate). Scales:
  gate = gate_int * c_g  (c_g = absmax_x * mean|wg| / 127, per token)
  c_u cancels inside the second act-quant; `up` stays integer
  out  = down_int * F_t  (F_t = maxg * absmax_x * mean|wu| mean|wd| / 127^2)
"""

import numpy as np

import concourse.bacc as bacc
import concourse.bass as bass
import concourse.bass_isa as bass_isa
import concourse.mybir as mybir
import concourse.tile as tile
from concourse.masks import make_identity

P = 128
HID = 1024
INNER = 4096
N_CORES = 8
T_CORE = 1024          # tokens per core
TC = 256               # token chunk in the main loop
NCH = T_CORE // TC     # 4 chunks
MT = TC // P           # 2 token tiles per chunk
KI = HID // P          # 8 contraction tiles for gate/up
KOG = INNER // P       # 32 contraction tiles for down
OSH = INNER // N_CORES  # 512, o-shard per core

MROUND = 12582912.0    # 1.5 * 2**23: (v + M) - M == round-half-even(v)
W_ELEMS = float(INNER * HID)

F32 = mybir.dt.float32
BF16 = mybir.dt.bfloat16
FP16 = mybir.dt.float16
FP8 = mybir.dt.float8e4

A = mybir.AluOpType
AF = mybir.ActivationFunctionType


def build_bass(sim_mode: bool = False, main_chunks: int = NCH, reps: int = 1):
    """Build the SPMD program. sim_mode replaces collectives with local
    stand-ins so the single-core cost-model simulator can run it."""
    nc = bacc.Bacc(
        "TRN2", target_bir_lowering=False, debug=False,
        num_devices=N_CORES,
    )
    groups = [list(range(N_CORES))]

    xT_d = nc.dram_tensor("xT_shard", [HID, T_CORE], F32, kind="ExternalInput")
    wg_d = nc.dram_tensor("wgT_shard", [HID, OSH], F32, kind="ExternalInput")
    wu_d = nc.dram_tensor("wuT_shard", [HID, OSH], F32, kind="ExternalInput")
    wd_d = nc.dram_tensor("wdT_shard", [OSH, HID], F32, kind="ExternalInput")
    out_d = nc.dram_tensor("out_shard", [T_CORE, HID], F32, kind="ExternalOutput")

    xT_r = xT_d.ap().rearrange("(ki p) t -> ki p t", p=P)     # [8, 128, 1024]
    wg_r = wg_d.ap().rearrange("(ki p) o -> ki p o", p=P)     # [8, 128, 512]
    wu_r = wu_d.ap().rearrange("(ki p) o -> ki p o", p=P)
    wd_r = wd_d.ap().rearrange("(ko p) h -> ko p h", p=P)     # [4, 128, 1024]
    out_r = out_d.ap().rearrange("(n p) h -> n p h", p=P)     # [8, 128, 1024]

    with tile.TileContext(nc) as tc:
        with (
            tc.tile_pool(name="const", bufs=1) as constp,
            tc.tile_pool(name="big", bufs=1) as bigp,
            tc.tile_pool(name="wstr", bufs=4) as wstrp,
            tc.tile_pool(name="xstr", bufs=2) as xstrp,
            tc.tile_pool(name="stg", bufs=1) as stgp,
            tc.tile_pool(name="ew", bufs=2) as ewp,
            tc.tile_pool(name="outp", bufs=2) as outpp,
            tc.tile_pool(name="tiny", bufs=2) as tinyp,
            tc.tile_pool(name="pg", bufs=5, space="PSUM") as pgp,
            tc.tile_pool(name="pd", bufs=2, space="PSUM") as pdp,
            tc.tile_pool(name="pt", bufs=2, space="PSUM") as ptp,
            tc.tile_pool(name="dram", bufs=1, space="DRAM") as dramp,
        ):
            ones_col = constp.tile([P, 1], F32)
            nc.gpsimd.memset(ones_col[:], 1.0)
            ones_row = constp.tile([1, P], F32)
            nc.gpsimd.memset(ones_row[:], 1.0)
            identf = constp.tile([P, P], F32)
            make_identity(nc, identf)

            def emit_body():
                dmaq = [nc.sync, nc.scalar]

                # ---------- pass 1: |w| partial sums over this core's shards
                sums_col = constp.tile([P, 4], F32)
                for j, (src, n_sub, w_, tag) in enumerate((
                    (wg_r, KI, OSH, "ws512"), (wu_r, KI, OSH, "ws512"),
                    (wd_r, 4, HID, "ws1024"),
                )):
                    for t in range(n_sub):
                        wld = wstrp.tile([P, w_], F32, tag=tag, bufs=4 if tag == "ws512" else 3)
                        dmaq[t % 2].dma_start(out=wld[:], in_=src[t])
                        if t == 0:
                            nc.vector.tensor_reduce(
                                out=sums_col[:, j:j + 1], in_=wld[:],
                                axis=mybir.AxisListType.X,
                                op=A.add, apply_absolute_value=True)
                        else:
                            part = tinyp.tile([P, 1], F32, tag="wabs")
                            nc.vector.tensor_reduce(
                                out=part[:], in_=wld[:],
                                axis=mybir.AxisListType.X,
                                op=A.add, apply_absolute_value=True)
                            nc.vector.tensor_tensor(
                                out=sums_col[:, j:j + 1],
                                in0=sums_col[:, j:j + 1],
                                in1=part[:], op=A.add)

                psums = ptp.tile([P, P], F32, tag="tr", bufs=1)
                nc.tensor.matmul(psums[0:1, 0:4], lhsT=ones_col[:],
                                 rhs=sums_col[:], start=True, stop=True)
                sums_sb = tinyp.tile([1, 4], F32)
                nc.vector.tensor_copy(out=sums_sb[:], in_=psums[0:1, 0:4])

                # tiny AllReduce of the three |w| sums
                sums_in = dramp.tile([1, 4], F32)
                sums_out = dramp.tile([1, 4], F32, addr_space="Shared")
                nc.sync.dma_start(out=sums_in[:], in_=sums_sb[:])
                if sim_mode:
                    nc.sync.dma_start(out=sums_out[:], in_=sums_in[:])
                else:
                    nc.gpsimd.collective_compute(
                        "AllReduce", A.add, replica_groups=groups,
                        ins=[sums_in[:]], outs=[sums_out[:]])
                sums_all = tinyp.tile([1, 4], F32)
                nc.sync.dma_start(out=sums_all[:], in_=sums_out[:])

                # ---------- scales from the AllReduced sums
                mclsw = tinyp.tile([1, 8], F32)
                nc.vector.tensor_scalar(
                    out=mclsw[0:1, 0:4], in0=sums_all[:], scalar1=1.0 / W_ELEMS,
                    scalar2=1e-5, op0=A.mult, op1=A.max)
                nc.vector.reciprocal(out=mclsw[0:1, 4:8], in_=mclsw[0:1, 0:4])
                mclsw_d = dramp.tile([1, 8], F32)
                nc.sync.dma_start(out=mclsw_d[:], in_=mclsw[:])
                mclswb = constp.tile([P, 8], F32)
                nc.sync.dma_start(
                    out=mclswb[:], in_=mclsw_d[:].to_broadcast((P, 8)))
                mclb = mclswb[:, 0:4]
                swb = mclswb[:, 4:8]
                # bc_ud = clip_mean_wu * clip_mean_wd / 127^2   (for F_t)
                bc_ud = constp.tile([P, 1], F32)
                nc.vector.tensor_tensor(
                    out=bc_ud[:], in0=mclswb[:, 1:2], in1=mclswb[:, 2:3],
                    op=A.mult)
                nc.vector.tensor_scalar_mul(bc_ud[:], bc_ud[:], 1.0 / (127.0 * 127.0))

                # ---------- pass 2: quantize weight shards -> fp8 staging -> AG
                shared_as = "Local" if sim_mode else "Shared"
                stg_wga = dramp.tile([KI * P * (OSH // 2)], FP8)
                stg_wgb = dramp.tile([KI * P * (OSH // 2)], FP8)
                stg_wu = dramp.tile([KI * P * OSH], FP8)
                stg_wd = dramp.tile([4 * P * HID], FP8)
                agt_wga = dramp.tile([N_CORES, KI * P * (OSH // 2)], FP8,
                                     addr_space=shared_as)
                agt_wgb = dramp.tile([N_CORES, KI * P * (OSH // 2)], FP8,
                                     addr_space=shared_as)
                agt_wu = dramp.tile([N_CORES, KI * P * OSH], FP8, addr_space=shared_as)
                agt_wd = dramp.tile([N_CORES, 4 * P * HID], FP8, addr_space=shared_as)

                def do_ag(stg, ag):
                    if sim_mode:
                        # timing stand-in only (no_exec sim): one local copy
                        # approximates the trigger; the real AG runs on
                        # TOPSP+SDMA off the compute engines.
                        nc.sync.dma_start(out=ag[0, :], in_=stg[:])
                        return
                    else:
                        nc.gpsimd.collective_compute(
                            "AllGather", A.bypass, replica_groups=groups,
                            ins=[stg[:]], outs=[ag[:]])

                def quant_tile(src_sb, dst_sb, w_idx, width, eng=None):
                    """dst fp8 <- clip(round(src * s_w), -1, 1); src clobbered."""
                    eng = eng or nc.vector
                    sw_col = swb[:, w_idx:w_idx + 1]
                    eng.tensor_scalar(
                        out=src_sb, in0=src_sb, scalar1=sw_col, scalar2=MROUND,
                        op0=A.mult, op1=A.add)
                    eng.tensor_scalar(
                        out=src_sb, in0=src_sb, scalar1=-MROUND,
                        scalar2=1.0, op0=A.add, op1=A.min)
                    eng.tensor_scalar(
                        out=dst_sb, in0=src_sb, scalar1=-1.0, scalar2=None,
                        op0=A.max)

                # wg first (gates the main loop), then x requant, wu, wd
                stage_g = stgp.tile([P, KI, OSH], FP8, tag="stg8", bufs=1)
                for t in range(KI):
                    wld = wstrp.tile([P, OSH], F32, tag="ws512", bufs=4)
                    dmaq[t % 2].dma_start(out=wld[:], in_=wg_r[t])
                    quant_tile(wld[:], stage_g[:, t, :], 0, OSH)
                oh = OSH // 2
                nc.sync.dma_start(
                    out=stg_wga[:].rearrange(
                        "(ki p o) -> p ki o", ki=KI, p=P, o=oh),
                    in_=stage_g[:, :, 0:oh])
                do_ag(stg_wga, agt_wga)
                nc.sync.dma_start(
                    out=stg_wgb[:].rearrange(
                        "(ki p o) -> p ki o", ki=KI, p=P, o=oh),
                    in_=stage_g[:, :, oh:OSH])
                do_ag(stg_wgb, agt_wgb)

                stage_u = stgp.tile([P, KI, OSH], FP8, tag="stg8", bufs=1)
                for t in range(KI):
                    wld = wstrp.tile([P, OSH], F32, tag="ws512", bufs=4)
                    dmaq[t % 2].dma_start(out=wld[:], in_=wu_r[t])
                    quant_tile(wld[:], stage_u[:, t, :], 1, OSH)
                nc.sync.dma_start(
                    out=stg_wu[:].rearrange(
                        "(ki p o) -> p ki o", ki=KI, p=P, o=OSH),
                    in_=stage_u[:])
                do_ag(stg_wu, agt_wu)

                # ---------- x pass 1: per-token absmax via partition reduce
                amr = constp.tile([P, T_CORE], F32)     # clip(absmax_x), bcast
                xqT = bigp.tile([P, KI, T_CORE], BF16, tag="xqT")
                for ts in range(KI):
                    xld = xstrp.tile([P, T_CORE], F32, tag="xld")
                    nc.scalar.dma_start(out=xld[:], in_=xT_r[ts])
                    nc.scalar.activation(xld[:], xld[:], AF.Abs)
                    nc.vector.tensor_tensor(
                        out=amr[:], in0=(xld if ts == 0 else amr)[:],
                        in1=xld[:], op=A.max)

                # reduce over partitions via PE transpose + free-dim max
                absm_c = constp.tile([P, KI], F32)
                for ts in range(KI):
                    trp = ptp.tile([P, P], F32, tag="tr", bufs=1)
                    nc.tensor.transpose(
                        trp[:], amr[:, ts * P:(ts + 1) * P], identf[:])
                    nc.vector.tensor_reduce(
                        out=absm_c[:, ts:ts + 1], in_=trp[:],
                        axis=mybir.AxisListType.X, op=A.max)
                nc.vector.tensor_scalar_max(absm_c[:], absm_c[:], 1e-5)
                # columns -> row staging, then PE-broadcast back into amr
                s1b = ewp.tile([P, T_CORE], F32, tag="s1b", bufs=1)
                for ts in range(KI):
                    nc.scalar.dma_start(
                        out=s1b[0:1, ts * P:(ts + 1) * P],
                        in_=absm_c[:, ts:ts + 1])
                am_d = dramp.tile([1, T_CORE], F32)
                nc.scalar.dma_start(out=am_d[:], in_=s1b[0:1, :])
                nc.scalar.dma_start(
                    out=amr[:], in_=am_d[:].to_broadcast((P, T_CORE)))
                nc.vector.reciprocal(out=s1b[:], in_=amr[:])
                nc.vector.tensor_scalar_mul(s1b[:], s1b[:], 127.0)
                # x pass 2: re-read f32, quantize -> xqT (bf16 ints)
                for ts in range(KI):
                    xq2 = xstrp.tile([P, T_CORE], F32, tag="xld")
                    nc.scalar.dma_start(out=xq2[:], in_=xT_r[ts])
                    nc.vector.tensor_tensor(
                        out=xq2[:], in0=xq2[:], in1=s1b[:], op=A.mult)
                    nc.vector.tensor_scalar(
                        out=xqT[:, ts, :], in0=xq2[:], scalar1=MROUND,
                        scalar2=-MROUND, op0=A.add, op1=A.add)

                stage_d = stgp.tile([P, 4, HID], FP8, tag="stgd")
                for t in range(4):
                    wld = wstrp.tile([P, HID], F32, tag="ws1024", bufs=3)
                    nc.scalar.dma_start(out=wld[:], in_=wd_r[t])
                    quant_tile(wld[:], stage_d[:, t, :], 2, HID)
                nc.scalar.dma_start(
                    out=stg_wd[:].rearrange(
                        "(ko p h) -> p ko h", ko=4, p=P, h=HID),
                    in_=stage_d[:])
                do_ag(stg_wd, agt_wd)

                # ---------- load gathered weights into SBUF caches
                wg_sb = bigp.tile([P, KI, INNER], FP8, tag="wgc")
                wu_sb = bigp.tile([P, KI, INNER], FP8, tag="wuc")
                wdt = bigp.tile([P, KOG, HID], FP8, tag="wdc")
                for c in range(N_CORES):
                    c0 = c * OSH
                    nc.sync.dma_start(
                        out=wg_sb[:, :, c0:c0 + oh],
                        in_=agt_wga[c, :].rearrange(
                            "(ki p o) -> p ki o", ki=KI, p=P, o=oh))
                for c in range(N_CORES):
                    c0 = c * OSH
                    nc.sync.dma_start(
                        out=wg_sb[:, :, c0 + oh:c0 + OSH],
                        in_=agt_wgb[c, :].rearrange(
                            "(ki p o) -> p ki o", ki=KI, p=P, o=oh))
                for c in range(N_CORES):
                    csl = slice(c * OSH, (c + 1) * OSH)
                    nc.sync.dma_start(
                        out=wu_sb[:, :, csl],
                        in_=agt_wu[c, :].rearrange(
                            "(ki p o) -> p ki o", ki=KI, p=P, o=OSH))
                for c in range(N_CORES):
                    nc.sync.dma_start(
                        out=wdt[:, c * 4:(c + 1) * 4, :],
                        in_=agt_wd[c, :].rearrange(
                            "(ko p h) -> p ko h", ko=4, p=P, h=HID))

                # ---------- main loop over token chunks (sw-pipelined) -------
                prods = [
                    bigp.tile([P, KOG, TC], FP16, tag=f"prod{i}",
                              name=f"prod{i}")
                    for i in range(2)
                ]

                def gu_phase(ch):
                    tsl = slice(ch * TC, (ch + 1) * TC)
                    prod = prods[ch % 2]
                    # c_g broadcast tile for this chunk
                    cgb = ewp.tile([P, TC], F32, tag="cgb")
                    nc.vector.tensor_scalar(
                        out=cgb[:], in0=amr[:, tsl], scalar1=mclb[:, 0:1],
                        scalar2=1.0 / 127.0, op0=A.mult, op1=A.mult)

                    # gate: silu(gate_int * c_g) -> prod (fp16)
                    m_order = [4 * c + r for c in range(N_CORES) for r in (0, 1)]
                    m_order += [4 * c + r for c in range(N_CORES) for r in (2, 3)]
                    for m in m_order:
                        osl = slice(m * P, (m + 1) * P)
                        psg = pgp.tile([P, TC], F32, tag="pg")
                        for ki in range(KI):
                            nc.tensor.matmul(
                                psg[:], lhsT=wg_sb[:, ki, osl],
                                rhs=xqT[:, ki, tsl],
                                start=(ki == 0), stop=(ki == KI - 1))
                        nc.vector.tensor_tensor(
                            out=psg[:], in0=psg[:], in1=cgb[:], op=A.mult)
                        nc.scalar.activation(prod[:, m], psg[:], AF.Silu)

                    # up: prod *= up_int; running per-token absmax
                    maxr = ewp.tile([P, TC], F32, tag="maxr")
                    for m in range(KOG):
                        osl = slice(m * P, (m + 1) * P)
                        psu = pgp.tile([P, TC], F32, tag="pg")
                        for ki in range(KI):
                            nc.tensor.matmul(
                                psu[:], lhsT=wu_sb[:, ki, osl],
                                rhs=xqT[:, ki, tsl],
                                start=(ki == 0), stop=(ki == KI - 1))
                        nc.vector.tensor_tensor(
                            out=prod[:, m], in0=prod[:, m], in1=psu[:],
                            op=A.mult)
                        pab = ewp.tile([P, TC], FP16, tag="pab")
                        nc.scalar.activation(pab[:], prod[:, m], AF.Abs)
                        nc.vector.tensor_tensor(
                            out=maxr[:], in0=(pab if m == 0 else maxr)[:],
                            in1=pab[:], op=A.max)

                    # second act-quant scale (gpsimd partition reduce)
                    maxg = ewp.tile([P, TC], F32, tag="maxg")
                    nc.gpsimd.partition_all_reduce(
                        maxg[:], maxr[:], channels=P,
                        reduce_op=bass_isa.ReduceOp.max)
                    nc.vector.tensor_scalar_max(maxg[:], maxg[:], 1e-5)
                    s2b = ewp.tile([P, TC], F32, tag="s2b")
                    nc.vector.reciprocal(out=s2b[:], in_=maxg[:])
                    nc.vector.tensor_scalar_mul(s2b[:], s2b[:], 127.0)
                    mgc = tinyp.tile([P, MT], F32, tag="mgc")
                    for mt in range(MT):
                        nc.sync.dma_start(
                            out=mgc[:, mt:mt + 1],
                            in_=maxg[0:1, mt * P:(mt + 1) * P])

                    # quantize prod in place (fp16 ints in [-127, 127])
                    for g in range(0, KOG, 2):
                        qt = ewp.tile([P, 2, TC], F32, tag="qtmp", bufs=1)
                        nc.vector.tensor_tensor(
                            out=qt[:], in0=prod[:, g:g + 2],
                            in1=s2b[:, None, :].to_broadcast((P, 2, TC)),
                            op=A.mult)
                        nc.vector.tensor_scalar(
                            out=prod[:, g:g + 2], in0=qt[:], scalar1=MROUND,
                            scalar2=-MROUND, op0=A.add, op1=A.add)

                    # F_t column form for this chunk
                    fcol = tinyp.tile([P, MT], F32, tag="fcol")
                    nc.vector.tensor_tensor(
                        out=fcol[:], in0=mgc[:],
                        in1=absm_c[:, ch * MT:(ch + 1) * MT], op=A.mult)
                    nc.vector.tensor_scalar_mul(fcol[:], fcol[:], bc_ud[:, 0:1])
                    return fcol

                def down_phase(ch, fcol):
                    prod = prods[ch % 2]
                    for mt in range(MT):
                        t0 = mt * P
                        for hh in range(2):
                            hsl = slice(hh * 512, (hh + 1) * 512)
                            psd = pdp.tile([P, 512], F32, tag="pd")
                            for kog in range(KOG):
                                nc.tensor.matmul(
                                    psd[:], lhsT=prod[:, kog, t0:t0 + P],
                                    rhs=wdt[:, kog, hsl],
                                    start=(kog == 0), stop=(kog == KOG - 1))
                            osb = outpp.tile([P, 512], F32, tag="osb")
                            nc.scalar.activation(
                                osb[:], psd[:], AF.Copy,
                                scale=fcol[:, mt:mt + 1])
                            nc.sync.dma_start(
                                out=out_r[ch * MT + mt][:, hsl], in_=osb[:])

                fcols = {}
                for ch in range(main_chunks):
                    fcols[ch] = gu_phase(ch)
                    if ch > 0:
                        down_phase(ch - 1, fcols[ch - 1])
                if main_chunks > 0:
                    down_phase(main_chunks - 1, fcols[main_chunks - 1])

            for _rep in range(reps):
                emit_body()

    nc.compile()
    return nc


_NC_CACHE = {}


def _get_nc():
    if "nc" not in _NC_CACHE:
        _NC_CACHE["nc"] = build_bass(sim_mode=False)
    return _NC_CACHE["nc"]


def make_in_maps(x, w_gate, w_up, w_down):
    x2 = np.asarray(x, dtype=np.float32).reshape(N_CORES * T_CORE, HID)
    wg = np.asarray(w_gate, dtype=np.float32)
    wu = np.asarray(w_up, dtype=np.float32)
    wd = np.asarray(w_down, dtype=np.float32)
    in_maps = []
    for c in range(N_CORES):
        in_maps.append({
            "xT_shard": np.ascontiguousarray(
                x2[c * T_CORE:(c + 1) * T_CORE].T),
            "wgT_shard": np.ascontiguousarray(
                wg[c * OSH:(c + 1) * OSH].T),
            "wuT_shard": np.ascontiguousarray(
                wu[c * OSH:(c + 1) * OSH].T),
            "wdT_shard": np.ascontiguousarray(
                wd[:, c * OSH:(c + 1) * OSH].T),
        })
    return in_maps


def assemble_output(results):
    parts = [results[c]["out_shard"] for c in range(N_CORES)]
    return np.concatenate(parts, axis=0).reshape(4, 2048, HID)


def kernel(x, w_gate, w_up, w_down):
    from concourse.bass_utils import run_bass_kernel_spmd
    nc = _get_nc()
    in_maps = make_in_maps(x, w_gate, w_up, w_down)
    res = run_bass_kernel_spmd(nc, in_maps, list(range(N_CORES)), trace=False)
    return assemble_output(res.results)


# revision 26
# speedup vs baseline: 1.3456x; 1.3456x over previous
"""BitLinear FFN (BitNet b1.58) Trainium2 kernel, 8-core SPMD — v2.

Strategy: data-parallel over tokens (1024 tokens/core). Host passes
pre-transposed inputs (xT, wgT, wuT, wdT) so no PE transposes are needed.
Weight quantization is sharded 1/8 per core (two streamed passes: |w| sums,
then quantize->fp8 into AllGather staging). Three pipelined AllGathers
(wg -> wu -> wd) overlap the main loop: chunk-0 gate matmuls start once wg
lands; the up phase covers the wu AG; the down phase covers the wd AG.

Exactness: act quant produces ints in [-127,127] (exact bf16); weights are
ternary fp8e4. PE accumulates fp32 -> integer-exact matmuls. The silu'd
gate and the gate*up product are carried in fp16 (rel err ~2^-11, well under
the 2e-2 gate). Scales:
  gate = gate_int * c_g  (c_g = absmax_x * mean|wg| / 127, per token)
  c_u cancels inside the second act-quant; `up` stays integer
  out  = down_int * F_t  (F_t = maxg * absmax_x * mean|wu| mean|wd| / 127^2)
"""

import numpy as np

import concourse.bacc as bacc
import concourse.bass as bass
import concourse.bass_isa as bass_isa
import concourse.mybir as mybir
import concourse.tile as tile
from concourse.masks import make_identity

P = 128
HID = 1024
INNER = 4096
N_CORES = 8
T_CORE = 1024          # tokens per core
TC = 256               # token chunk in the main loop
NCH = T_CORE // TC     # 4 chunks
MT = TC // P           # 2 token tiles per chunk
KI = HID // P          # 8 contraction tiles for gate/up
KOG = INNER // P       # 32 contraction tiles for down
OSH = INNER // N_CORES  # 512, o-shard per core

MROUND = 12582912.0    # 1.5 * 2**23: (v + M) - M == round-half-even(v)
W_ELEMS = float(INNER * HID)

F32 = mybir.dt.float32
BF16 = mybir.dt.bfloat16
FP16 = mybir.dt.float16
FP8 = mybir.dt.float8e4

A = mybir.AluOpType
AF = mybir.ActivationFunctionType


def build_bass(sim_mode: bool = False, main_chunks: int = NCH, reps: int = 1):
    """Build the SPMD program. sim_mode replaces collectives with local
    stand-ins so the single-core cost-model simulator can run it."""
    nc = bacc.Bacc(
        "TRN2", target_bir_lowering=False, debug=False,
        num_devices=N_CORES,
    )
    groups = [list(range(N_CORES))]

    xT_d = nc.dram_tensor("xT_shard", [HID, T_CORE], F32, kind="ExternalInput")
    wg_d = nc.dram_tensor("wgT_shard", [HID, OSH], F32, kind="ExternalInput")
    wu_d = nc.dram_tensor("wuT_shard", [HID, OSH], F32, kind="ExternalInput")
    wd_d = nc.dram_tensor("wdT_shard", [OSH, HID], F32, kind="ExternalInput")
    out_d = nc.dram_tensor("out_shard", [T_CORE, HID], F32, kind="ExternalOutput")

    xT_r = xT_d.ap().rearrange("(ki p) t -> ki p t", p=P)     # [8, 128, 1024]
    wg_r = wg_d.ap().rearrange("(ki p) o -> ki p o", p=P)     # [8, 128, 512]
    wu_r = wu_d.ap().rearrange("(ki p) o -> ki p o", p=P)
    wd_r = wd_d.ap().rearrange("(ko p) h -> ko p h", p=P)     # [4, 128, 1024]
    out_r = out_d.ap().rearrange("(n p) h -> n p h", p=P)     # [8, 128, 1024]

    with tile.TileContext(nc) as tc:
        with (
            tc.tile_pool(name="const", bufs=1) as constp,
            tc.tile_pool(name="big", bufs=1) as bigp,
            tc.tile_pool(name="wstr", bufs=4) as wstrp,
            tc.tile_pool(name="xstr", bufs=2) as xstrp,
            tc.tile_pool(name="stg", bufs=1) as stgp,
            tc.tile_pool(name="ew", bufs=2) as ewp,
            tc.tile_pool(name="outp", bufs=2) as outpp,
            tc.tile_pool(name="tiny", bufs=2) as tinyp,
            tc.tile_pool(name="pg", bufs=5, space="PSUM") as pgp,
            tc.tile_pool(name="pd", bufs=2, space="PSUM") as pdp,
            tc.tile_pool(name="pt", bufs=2, space="PSUM") as ptp,
            tc.tile_pool(name="dram", bufs=1, space="DRAM") as dramp,
        ):
            ones_col = constp.tile([P, 1], F32)
            nc.gpsimd.memset(ones_col[:], 1.0)
            ones_row = constp.tile([1, P], F32)
            nc.gpsimd.memset(ones_row[:], 1.0)
            identf = constp.tile([P, P], F32)
            make_identity(nc, identf)

            def emit_body():
                dmaq = [nc.sync, nc.scalar]

                # ---------- pass 1: |w| partial sums over this core's shards
                sums_col = constp.tile([P, 4], F32)
                for j, (src, n_sub, w_, tag) in enumerate((
                    (wg_r, KI, OSH, "ws512"), (wu_r, KI, OSH, "ws512"),
                    (wd_r, 4, HID, "ws1024"),
                )):
                    for t in range(n_sub):
                        wld = wstrp.tile([P, w_], F32, tag=tag, bufs=4 if tag == "ws512" else 3)
                        dmaq[t % 2].dma_start(out=wld[:], in_=src[t])
                        if t == 0:
                            nc.vector.tensor_reduce(
                                out=sums_col[:, j:j + 1], in_=wld[:],
                                axis=mybir.AxisListType.X,
                                op=A.add, apply_absolute_value=True)
                        else:
                            part = tinyp.tile([P, 1], F32, tag="wabs")
                            nc.vector.tensor_reduce(
                                out=part[:], in_=wld[:],
                                axis=mybir.AxisListType.X,
                                op=A.add, apply_absolute_value=True)
                            nc.vector.tensor_tensor(
                                out=sums_col[:, j:j + 1],
                                in0=sums_col[:, j:j + 1],
                                in1=part[:], op=A.add)

                psums = ptp.tile([P, P], F32, tag="tr", bufs=1)
                nc.tensor.matmul(psums[0:1, 0:4], lhsT=ones_col[:],
                                 rhs=sums_col[:], start=True, stop=True)
                sums_sb = tinyp.tile([1, 4], F32)
                nc.vector.tensor_copy(out=sums_sb[:], in_=psums[0:1, 0:4])

                # tiny AllReduce of the three |w| sums
                sums_in = dramp.tile([1, 4], F32)
                sums_out = dramp.tile([1, 4], F32, addr_space="Shared")
                nc.sync.dma_start(out=sums_in[:], in_=sums_sb[:])
                if sim_mode:
                    nc.sync.dma_start(out=sums_out[:], in_=sums_in[:])
                else:
                    nc.gpsimd.collective_compute(
                        "AllReduce", A.add, replica_groups=groups,
                        ins=[sums_in[:]], outs=[sums_out[:]])
                sums_all = tinyp.tile([1, 4], F32)
                nc.sync.dma_start(out=sums_all[:], in_=sums_out[:])

                # ---------- scales from the AllReduced sums
                mclsw = tinyp.tile([1, 8], F32)
                nc.vector.tensor_scalar(
                    out=mclsw[0:1, 0:4], in0=sums_all[:], scalar1=1.0 / W_ELEMS,
                    scalar2=1e-5, op0=A.mult, op1=A.max)
                nc.vector.reciprocal(out=mclsw[0:1, 4:8], in_=mclsw[0:1, 0:4])
                mclsw_d = dramp.tile([1, 8], F32)
                nc.sync.dma_start(out=mclsw_d[:], in_=mclsw[:])
                mclswb = constp.tile([P, 8], F32)
                nc.sync.dma_start(
                    out=mclswb[:], in_=mclsw_d[:].to_broadcast((P, 8)))
                mclb = mclswb[:, 0:4]
                swb = mclswb[:, 4:8]
                # bc_ud = clip_mean_wu * clip_mean_wd / 127^2   (for F_t)
                bc_ud = constp.tile([P, 1], F32)
                nc.vector.tensor_tensor(
                    out=bc_ud[:], in0=mclswb[:, 1:2], in1=mclswb[:, 2:3],
                    op=A.mult)
                nc.vector.tensor_scalar_mul(bc_ud[:], bc_ud[:], 1.0 / (127.0 * 127.0))

                # ---------- pass 2: quantize weight shards -> fp8 staging -> AG
                shared_as = "Local" if sim_mode else "Shared"
                stg_wga = dramp.tile([KI * P * (OSH // 2)], FP8)
                stg_wgb = dramp.tile([KI * P * (OSH // 2)], FP8)
                stg_wu = dramp.tile([KI * P * OSH], FP8)
                stg_wd = dramp.tile([4 * P * HID], FP8)
                agt_wga = dramp.tile([N_CORES, KI * P * (OSH // 2)], FP8,
                                     addr_space=shared_as)
                agt_wgb = dramp.tile([N_CORES, KI * P * (OSH // 2)], FP8,
                                     addr_space=shared_as)
                agt_wu = dramp.tile([N_CORES, KI * P * OSH], FP8, addr_space=shared_as)
                agt_wd = dramp.tile([N_CORES, 4 * P * HID], FP8, addr_space=shared_as)

                def do_ag(stg, ag):
                    if sim_mode:
                        # timing stand-in only (no_exec sim): one local copy
                        # approximates the trigger; the real AG runs on
                        # TOPSP+SDMA off the compute engines.
                        nc.sync.dma_start(out=ag[0, :], in_=stg[:])
                        return
                    else:
                        nc.gpsimd.collective_compute(
                            "AllGather", A.bypass, replica_groups=groups,
                            ins=[stg[:]], outs=[ag[:]])

                def quant_tile(src_sb, dst_sb, w_idx, width, eng=None):
                    """dst fp8 <- clip(round(src * s_w), -1, 1); src clobbered."""
                    eng = eng or nc.vector
                    sw_col = swb[:, w_idx:w_idx + 1]
                    eng.tensor_scalar(
                        out=src_sb, in0=src_sb, scalar1=sw_col, scalar2=MROUND,
                        op0=A.mult, op1=A.add)
                    eng.tensor_scalar(
                        out=src_sb, in0=src_sb, scalar1=-MROUND,
                        scalar2=1.0, op0=A.add, op1=A.min)
                    eng.tensor_scalar(
                        out=dst_sb, in0=src_sb, scalar1=-1.0, scalar2=None,
                        op0=A.max)

                # wg first (gates the main loop), then x requant, wu, wd
                stage_g = stgp.tile([P, KI, OSH], FP8, tag="stg8", bufs=1)
                for t in range(KI):
                    wld = wstrp.tile([P, OSH], F32, tag="ws512", bufs=4)
                    dmaq[t % 2].dma_start(out=wld[:], in_=wg_r[t])
                    quant_tile(wld[:], stage_g[:, t, :], 0, OSH)
                oh = OSH // 2
                nc.sync.dma_start(
                    out=stg_wga[:].rearrange(
                        "(ki p o) -> p ki o", ki=KI, p=P, o=oh),
                    in_=stage_g[:, :, 0:oh])
                do_ag(stg_wga, agt_wga)
                nc.sync.dma_start(
                    out=stg_wgb[:].rearrange(
                        "(ki p o) -> p ki o", ki=KI, p=P, o=oh),
                    in_=stage_g[:, :, oh:OSH])
                do_ag(stg_wgb, agt_wgb)

                stage_u = stgp.tile([P, KI, OSH], FP8, tag="stg8", bufs=1)
                for t in range(KI):
                    wld = wstrp.tile([P, OSH], F32, tag="ws512", bufs=4)
                    dmaq[t % 2].dma_start(out=wld[:], in_=wu_r[t])
                    quant_tile(wld[:], stage_u[:, t, :], 1, OSH)
                nc.sync.dma_start(
                    out=stg_wu[:].rearrange(
                        "(ki p o) -> p ki o", ki=KI, p=P, o=OSH),
                    in_=stage_u[:])
                do_ag(stg_wu, agt_wu)

                # ---------- x pass 1: per-token absmax via partition reduce
                amr = constp.tile([P, T_CORE], F32)     # clip(absmax_x), bcast
                xqT = bigp.tile([P, KI, T_CORE], BF16, tag="xqT")
                for ts in range(KI):
                    xld = xstrp.tile([P, T_CORE], F32, tag="xld")
                    nc.scalar.dma_start(out=xld[:], in_=xT_r[ts])
                    nc.scalar.activation(xld[:], xld[:], AF.Abs)
                    nc.vector.tensor_tensor(
                        out=amr[:], in0=(xld if ts == 0 else amr)[:],
                        in1=xld[:], op=A.max)

                # reduce over partitions via PE transpose + free-dim max
                absm_c = constp.tile([P, KI], F32)
                for ts in range(KI):
                    trp = ptp.tile([P, P], F32, tag="tr", bufs=1)
                    nc.tensor.transpose(
                        trp[:], amr[:, ts * P:(ts + 1) * P], identf[:])
                    nc.vector.tensor_reduce(
                        out=absm_c[:, ts:ts + 1], in_=trp[:],
                        axis=mybir.AxisListType.X, op=A.max)
                nc.vector.tensor_scalar_max(absm_c[:], absm_c[:], 1e-5)
                # columns -> row staging, then PE-broadcast back into amr
                s1b = ewp.tile([P, T_CORE], F32, tag="s1b", bufs=1)
                for ts in range(KI):
                    nc.scalar.dma_start(
                        out=s1b[0:1, ts * P:(ts + 1) * P],
                        in_=absm_c[:, ts:ts + 1])
                am_d = dramp.tile([1, T_CORE], F32)
                nc.scalar.dma_start(out=am_d[:], in_=s1b[0:1, :])
                nc.scalar.dma_start(
                    out=amr[:], in_=am_d[:].to_broadcast((P, T_CORE)))
                nc.vector.reciprocal(out=s1b[:], in_=amr[:])
                nc.vector.tensor_scalar_mul(s1b[:], s1b[:], 127.0)
                # x pass 2: re-read f32, quantize -> xqT (bf16 ints)
                for ts in range(KI):
                    xq2 = xstrp.tile([P, T_CORE], F32, tag="xld")
                    nc.scalar.dma_start(out=xq2[:], in_=xT_r[ts])
                    nc.vector.tensor_tensor(
                        out=xq2[:], in0=xq2[:], in1=s1b[:], op=A.mult)
                    nc.vector.tensor_scalar(
                        out=xqT[:, ts, :], in0=xq2[:], scalar1=MROUND,
                        scalar2=-MROUND, op0=A.add, op1=A.add)

                stage_d = stgp.tile([P, 4, HID], FP8, tag="stgd")
                for t in range(4):
                    wld = wstrp.tile([P, HID], F32, tag="ws1024", bufs=3)
                    nc.scalar.dma_start(out=wld[:], in_=wd_r[t])
                    quant_tile(wld[:], stage_d[:, t, :], 2, HID)
                nc.scalar.dma_start(
                    out=stg_wd[:].rearrange(
                        "(ko p h) -> p ko h", ko=4, p=P, h=HID),
                    in_=stage_d[:])
                do_ag(stg_wd, agt_wd)

                # ---------- load gathered weights into SBUF caches
                wg_sb = bigp.tile([P, KI, INNER], FP8, tag="wgc")
                wu_sb = bigp.tile([P, KI, INNER], FP8, tag="wuc")
                wdt = bigp.tile([P, KOG, HID], FP8, tag="wdc")
                for c in range(N_CORES):
                    c0 = c * OSH
                    nc.sync.dma_start(
                        out=wg_sb[:, :, c0:c0 + oh],
                        in_=agt_wga[c, :].rearrange(
                            "(ki p o) -> p ki o", ki=KI, p=P, o=oh))
                for c in range(N_CORES):
                    c0 = c * OSH
                    nc.sync.dma_start(
                        out=wg_sb[:, :, c0 + oh:c0 + OSH],
                        in_=agt_wgb[c, :].rearrange(
                            "(ki p o) -> p ki o", ki=KI, p=P, o=oh))
                for c in range(N_CORES):
                    csl = slice(c * OSH, (c + 1) * OSH)
                    nc.sync.dma_start(
                        out=wu_sb[:, :, csl],
                        in_=agt_wu[c, :].rearrange(
                            "(ki p o) -> p ki o", ki=KI, p=P, o=OSH))
                for c in range(N_CORES):
                    nc.sync.dma_start(
                        out=wdt[:, c * 4:(c + 1) * 4, :],
                        in_=agt_wd[c, :].rearrange(
                            "(ko p h) -> p ko h", ko=4, p=P, h=HID))

                # ---------- main loop over token chunks (sw-pipelined) -------
                prods = [
                    bigp.tile([P, KOG, TC], FP16, tag=f"prod{i}",
                              name=f"prod{i}")
                    for i in range(2)
                ]

                def gate_phase(ch):
                    tsl = slice(ch * TC, (ch + 1) * TC)
                    prod = prods[ch % 2]
                    # c_g broadcast tile for this chunk
                    cgb = ewp.tile([P, TC], F32, tag="cgb")
                    nc.vector.tensor_scalar(
                        out=cgb[:], in0=amr[:, tsl], scalar1=mclb[:, 0:1],
                        scalar2=1.0 / 127.0, op0=A.mult, op1=A.mult)

                    # gate: silu(gate_int * c_g) -> prod (fp16)
                    m_order = [4 * c + r for c in range(N_CORES) for r in (0, 1)]
                    m_order += [4 * c + r for c in range(N_CORES) for r in (2, 3)]
                    for m in m_order:
                        osl = slice(m * P, (m + 1) * P)
                        psg = pgp.tile([P, TC], F32, tag="pg")
                        for ki in range(KI):
                            nc.tensor.matmul(
                                psg[:], lhsT=wg_sb[:, ki, osl],
                                rhs=xqT[:, ki, tsl],
                                start=(ki == 0), stop=(ki == KI - 1))
                        nc.vector.tensor_tensor(
                            out=psg[:], in0=psg[:], in1=cgb[:], op=A.mult)
                        nc.scalar.activation(prod[:, m], psg[:], AF.Silu)

                def up_phase(ch):
                    tsl = slice(ch * TC, (ch + 1) * TC)
                    prod = prods[ch % 2]
                    # up: prod *= up_int; running per-token absmax
                    maxr = ewp.tile([P, TC], F32, tag="maxr")
                    for m in range(KOG):
                        osl = slice(m * P, (m + 1) * P)
                        psu = pgp.tile([P, TC], F32, tag="pg")
                        for ki in range(KI):
                            nc.tensor.matmul(
                                psu[:], lhsT=wu_sb[:, ki, osl],
                                rhs=xqT[:, ki, tsl],
                                start=(ki == 0), stop=(ki == KI - 1))
                        nc.vector.tensor_tensor(
                            out=prod[:, m], in0=prod[:, m], in1=psu[:],
                            op=A.mult)
                        pab = ewp.tile([P, TC], FP16, tag="pab")
                        nc.scalar.activation(pab[:], prod[:, m], AF.Abs)
                        nc.vector.tensor_tensor(
                            out=maxr[:], in0=(pab if m == 0 else maxr)[:],
                            in1=pab[:], op=A.max)

                    # second act-quant scale (gpsimd partition reduce)
                    maxg = ewp.tile([P, TC], F32, tag="maxg")
                    nc.gpsimd.partition_all_reduce(
                        maxg[:], maxr[:], channels=P,
                        reduce_op=bass_isa.ReduceOp.max)
                    nc.vector.tensor_scalar_max(maxg[:], maxg[:], 1e-5)
                    s2b = ewp.tile([P, TC], F32, tag="s2b")
                    nc.vector.reciprocal(out=s2b[:], in_=maxg[:])
                    nc.vector.tensor_scalar_mul(s2b[:], s2b[:], 127.0)
                    mgc = tinyp.tile([P, MT], F32, tag="mgc")
                    for mt in range(MT):
                        nc.sync.dma_start(
                            out=mgc[:, mt:mt + 1],
                            in_=maxg[0:1, mt * P:(mt + 1) * P])

                    # quantize prod in place (fp16 ints in [-127, 127])
                    for g in range(0, KOG, 2):
                        qt = ewp.tile([P, 2, TC], F32, tag="qtmp", bufs=1)
                        nc.vector.tensor_tensor(
                            out=qt[:], in0=prod[:, g:g + 2],
                            in1=s2b[:, None, :].to_broadcast((P, 2, TC)),
                            op=A.mult)
                        nc.vector.tensor_scalar(
                            out=prod[:, g:g + 2], in0=qt[:], scalar1=MROUND,
                            scalar2=-MROUND, op0=A.add, op1=A.add)

                    # F_t column form for this chunk
                    fcol = tinyp.tile([P, MT], F32, tag="fcol")
                    nc.vector.tensor_tensor(
                        out=fcol[:], in0=mgc[:],
                        in1=absm_c[:, ch * MT:(ch + 1) * MT], op=A.mult)
                    nc.vector.tensor_scalar_mul(fcol[:], fcol[:], bc_ud[:, 0:1])
                    return fcol

                def down_phase(ch, fcol):
                    prod = prods[ch % 2]
                    for mt in range(MT):
                        t0 = mt * P
                        for hh in range(2):
                            hsl = slice(hh * 512, (hh + 1) * 512)
                            psd = pdp.tile([P, 512], F32, tag="pd")
                            for kog in range(KOG):
                                nc.tensor.matmul(
                                    psd[:], lhsT=prod[:, kog, t0:t0 + P],
                                    rhs=wdt[:, kog, hsl],
                                    start=(kog == 0), stop=(kog == KOG - 1))
                            osb = outpp.tile([P, 512], F32, tag="osb")
                            nc.scalar.activation(
                                osb[:], psd[:], AF.Copy,
                                scale=fcol[:, mt:mt + 1])
                            nc.sync.dma_start(
                                out=out_r[ch * MT + mt][:, hsl], in_=osb[:])

                # pipelined emission: two gate phases up front so the PE
                # has wg-only work covering the wu AllGather arrival
                fcols = {}
                if main_chunks == NCH:
                    gate_phase(0)
                    gate_phase(1)
                    fcols[0] = up_phase(0)
                    fcols[1] = up_phase(1)
                    down_phase(0, fcols[0])
                    gate_phase(2)
                    fcols[2] = up_phase(2)
                    down_phase(1, fcols[1])
                    gate_phase(3)
                    fcols[3] = up_phase(3)
                    down_phase(2, fcols[2])
                    down_phase(3, fcols[3])
                else:
                    for ch in range(main_chunks):
                        gate_phase(ch)
                        fcols[ch] = up_phase(ch)
                        if ch > 0:
                            down_phase(ch - 1, fcols[ch - 1])
                    if main_chunks > 0:
                        down_phase(main_chunks - 1, fcols[main_chunks - 1])

            for _rep in range(reps):
                emit_body()

    nc.compile()
    return nc


_NC_CACHE = {}


def _get_nc():
    if "nc" not in _NC_CACHE:
        _NC_CACHE["nc"] = build_bass(sim_mode=False)
    return _NC_CACHE["nc"]


def make_in_maps(x, w_gate, w_up, w_down):
    x2 = np.asarray(x, dtype=np.float32).reshape(N_CORES * T_CORE, HID)
    wg = np.asarray(w_gate, dtype=np.float32)
    wu = np.asarray(w_up, dtype=np.float32)
    wd = np.asarray(w_down, dtype=np.float32)
    in_maps = []
    for c in range(N_CORES):
        in_maps.append({
            "xT_shard": np.ascontiguousarray(
                x2[c * T_CORE:(c + 1) * T_CORE].T),
            "wgT_shard": np.ascontiguousarray(
                wg[c * OSH:(c + 1) * OSH].T),
            "wuT_shard": np.ascontiguousarray(
                wu[c * OSH:(c + 1) * OSH].T),
            "wdT_shard": np.ascontiguousarray(
                wd[:, c * OSH:(c + 1) * OSH].T),
        })
    return in_maps


def assemble_output(results):
    parts = [results[c]["out_shard"] for c in range(N_CORES)]
    return np.concatenate(parts, axis=0).reshape(4, 2048, HID)


def kernel(x, w_gate, w_up, w_down):
    from concourse.bass_utils import run_bass_kernel_spmd
    nc = _get_nc()
    in_maps = make_in_maps(x, w_gate, w_up, w_down)
    res = run_bass_kernel_spmd(nc, in_maps, list(range(N_CORES)), trace=False)
    return assemble_output(res.results)
